# revision 36
# baseline (speedup 1.0000x reference)
"""Trainium2 Bass kernel for nn_HGNN_lstm (GNN message passing + LSTM).

Sharding: data-parallel over batch B=8 across 8 NeuronCores (one video per
core, zero collectives). Small weights replicated.

Math notes (exploits guaranteed input structure from setup_inputs):
  - edge_resnet is zero at invalid pairs, node_resnet zero at invalid nodes.
    Hence with gate >= 0 (sigmoid * mask):
      h_edge_{r+1} = where(pair_mask, gate*Msg, edge) == gate * Msg
      h_node_{r+1} = where(node_mask, h_new, node)    == node_mask * h_new
  - FORM-1 recurrence: materialize hE = gate*Msg directly (it is needed for
    m_v anyway). Phase A then consumes hE with no gate recurrence at all:
      adj_r = W2 @ relu(W1 @ hE_{r-1}) + b2      (exact: relu passes gate)
  - LSTM g-gate weights are pre-scaled by 2 on host so tanh(g) = 2*sig(2g)-1
    comes from the same single 4-gate sigmoid ACT op (exact identity).
  - The final GNN round is fused with the temporal LSTM: the GRU runs in
    8-frame chunks as soon as m_v for those frames lands, and LSTM steps are
    woven ~2 per M-tile iteration so the serial (~1.5us/step) LSTM chain
    hides under the round-2 message-passing compute.
"""

import sys
from contextlib import ExitStack

import numpy as np

sys.path.insert(0, "/opt/trn_rl_repo")

import concourse.bacc as bacc  # noqa: E402
import concourse.bass as bass  # noqa: E402
import concourse.mybir as mybir  # noqa: E402
import concourse.tile as tile  # noqa: E402
from concourse.bass_utils import run_bass_kernel_spmd  # noqa: E402

B, T, N, D = 8, 32, 24, 128
H_LINK, H_LSTM, C, P_ROUNDS = 128, 128, 6, 3
NP = N * N  # 576 pairs per frame
TNP = T * NP  # 18432
TN = T * N  # 768

F32 = mybir.dt.float32
BF16 = mybir.dt.bfloat16
FR = mybir.ActivationFunctionType
ALU = mybir.AluOpType
AX = mybir.AxisListType

import ml_dtypes  # noqa: E402

BULK_DT = BF16
BULK_NP = ml_dtypes.bfloat16


def _np_bulk(x):
    return np.ascontiguousarray(np.asarray(x).astype(BULK_NP))


# Const bundles: one bf16 + one f32 [128, cols] tensor, loaded with a single
# DMA trigger each (23 separate const DMAs queued the first weight behind
# ~1.5MB of edge chunks and delayed the first matmul to ~23us).
BF16_SPEC = [
    ("w1t", 128), ("w2pad", 64), ("wet", 128), ("wht", 128),
    ("gwihr", 128), ("gwihz", 128), ("gwihn", 128),
    ("gwhhr", 128), ("gwhhz", 128), ("gwhhn", 128),
    ("lwih", 512), ("lwhh", 512), ("rowt", 6),
]
F32_SPEC = [
    ("b1c", 1), ("b2c", 1), ("msgbc", 1), ("gbr", 1), ("gbz", 1),
    ("gbin", 1), ("gbhn", 1), ("lb4", 4), ("rob", 1),
]
F32M_SPEC = [
    ("fmaskp", 576), ("masknf", 768), ("maskro", 768),
]


def _spec_offsets(spec):
    out, o = {}, 0
    for name, w in spec:
        out[name] = (o, w)
        o += w
    return out, o


BF16_OFF, BF16_COLS = _spec_offsets(BF16_SPEC)
F32_OFF, F32_COLS = _spec_offsets(F32_SPEC)
F32M_OFF, F32M_COLS = _spec_offsets(F32M_SPEC)

_PROG_CACHE = {}


def _build_program(flags=()):
    use_lstm_bias = "lstm_bias" in flags
    use_msg_bias = "msg_bias" in flags

    nc = bacc.Bacc("TRN2", target_bir_lowering=False, debug=False)
    dt = BULK_DT

    def din(name, shape, d=dt):
        return nc.dram_tensor(name, shape, d, kind="ExternalInput").ap()

    edge = din("edge", [D, TNP])          # [d, t*576 + v*24 + w]
    node = din("node", [D, TN], F32)      # [d, t*24 + n]
    cbund = din("cbund", [D, BF16_COLS])  # bf16 const bundle
    fbund = din("fbund", [D, F32_COLS], F32)   # small f32 consts
    mbund = din("mbund", [D, F32M_COLS], F32)  # masks (late load)

    pred = nc.dram_tensor("pred", [C, TN], F32, kind="ExternalOutput").ap()

    with tile.TileContext(nc) as tc, ExitStack() as ctx:
        cp = ctx.enter_context(tc.tile_pool(name="consts", bufs=1))
        _dma_engines = [nc.sync, nc.scalar, nc.gpsimd]
        _dma_rr = [0]

        def dma_rr(dst, src):
            eng = _dma_engines[_dma_rr[0] % len(_dma_engines)]
            _dma_rr[0] += 1
            eng.dma_start(dst, src)

        big = ctx.enter_context(tc.tile_pool(name="big", bufs=1))
        E_all = big.tile([D, TNP], dt)
        hE_all = big.tile([D, TNP], dt)
        Hn_all = big.tile([D, TN], F32)
        Hn16 = big.tile([D, TN], BF16)
        mv_all = big.tile([D, TN], BF16)
        Hout_all = big.tile([D, TN], BF16)
        c_sb = big.tile([D, N], F32)

        cb = cp.tile([D, BF16_COLS], dt, name="cb")
        fb = cp.tile([D, F32_COLS], F32, name="fb")
        mb = cp.tile([D, F32M_COLS], F32, name="mb")
        # Small bundles + node first, then the 4.7MB edge sweep; the 1MB
        # mask bundle last (first needed at gate_0, ~25us in).
        dma_rr(cb[:], cbund)
        dma_rr(fb[:], fbund)
        dma_rr(Hn_all[:], node)
        for c in range(16):
            sl = slice(c * (TNP // 16), (c + 1) * (TNP // 16))
            dma_rr(E_all[:, sl], edge[:, sl])
        dma_rr(mb[:], mbund)

        def cs_(name):
            o, w = BF16_OFF[name]
            return cb[:, o:o + w]

        def fs_(name, parts=D):
            o, w = F32_OFF[name]
            return fb[0:parts, o:o + w]

        def ms_(name, parts=D):
            o, w = F32M_OFF[name]
            return mb[0:parts, o:o + w]

        w1t_s = cs_("w1t")
        w2pad_s = cs_("w2pad")
        wet_s = cs_("wet")
        wht_s = cs_("wht")
        gwihr_s, gwihz_s, gwihn_s = cs_("gwihr"), cs_("gwihz"), cs_("gwihn")
        gwhhr_s, gwhhz_s, gwhhn_s = cs_("gwhhr"), cs_("gwhhz"), cs_("gwhhn")
        lwih_s = cs_("lwih")
        lwhh_s = cs_("lwhh")
        rowt_s = cs_("rowt")
        b1c_s = fs_("b1c")
        b2c_s = fs_("b2c", T)
        msgbc_s = fs_("msgbc")
        gbr_s, gbz_s = fs_("gbr"), fs_("gbz")
        gbin_s, gbhn_s = fs_("gbin"), fs_("gbhn")
        lb4_s = fs_("lb4")
        rob_s = fs_("rob", C)
        fmaskp_s = ms_("fmaskp", T)
        masknf_s = ms_("masknf")
        maskro_s = ms_("maskro", C)

        apool = ctx.enter_context(tc.tile_pool(name="apool", bufs=3))
        bcpool = ctx.enter_context(tc.tile_pool(name="bcpool", bufs=3))
        trpool = ctx.enter_context(tc.tile_pool(name="trpool", bufs=3))
        gpool = ctx.enter_context(tc.tile_pool(name="gpool", bufs=2))
        utpool = ctx.enter_context(tc.tile_pool(name="utpool", bufs=1))
        lpool = ctx.enter_context(tc.tile_pool(name="lpool", bufs=2))
        gdram = ctx.enter_context(
            tc.tile_pool(name="gdram", bufs=2, space="DRAM"))

        def mm512(out_ps, lhsT, rhs, start, stop=False, base=0):
            """Matmul split into <=512-col chunks aligned to PSUM banks."""
            nfree = rhs.shape[-1]
            o = 0
            while o < nfree:
                sz = min(512 - ((base + o) % 512), nfree - o)
                nc.tensor.matmul(out_ps[:, base + o:base + o + sz], lhsT,
                                 rhs[:, o:o + sz],
                                 start=start, stop=stop and (o + sz >= nfree))
                o += sz

        def mh_mms(ps, base, hn_t):
            """Accumulate Wh@h broadcast over v into ps[:, base:base+576]."""
            pos = 0
            while pos < 576:
                room = 512 - ((base + pos) % 512)
                take = min(room, 576 - pos)
                while take > 0:
                    v, wofs = divmod(pos, N)
                    if wofs == 0 and take >= N:
                        nv = take // N
                        rhs = hn_t.rearrange("p (o w) -> p o w", o=1) \
                                  .broadcast_to([D, nv, N])
                        adv = nv * N
                    else:
                        adv = min(take, N - wofs)
                        rhs = hn_t[:, wofs:wofs + adv]
                    nc.tensor.matmul(ps[:, base + pos:base + pos + adv],
                                     wht_s, rhs, start=False, stop=False)
                    pos += adv
                    take -= adv

        def a_w1(tp, Xr, pspool):
            t0 = 2 * tp
            psA = pspool.tile([D, 2 * NP], F32, tag="big", bufs=2)
            mm512(psA, w1t_s, Xr[:, t0 * NP:(t0 + 2) * NP],
                  start=True, stop=True, base=0)
            return psA

        def a_relu(tp, psA, on_act):
            a_sb = apool.tile([D, 2 * NP], dt, tag="a")
            if on_act:
                nc.scalar.activation(a_sb[:], psA[:], FR.Relu,
                                     bias=b1c_s)
            else:
                nc.vector.tensor_scalar(a_sb[:], psA[:], b1c_s, 0.0,
                                        op0=ALU.add, op1=ALU.max)
            return a_sb

        def a_w2(tp, a_sb, psPall):
            t0 = 2 * tp
            for k in range(2):
                t = t0 + k
                mm512(psPall, w2pad_s[:, 32 - t:64 - t],
                      a_sb[:, k * NP:(k + 1) * NP],
                      start=(t == 0), stop=(t == T - 1))

        def phase_a0(pspool):
            """Standalone A_0 (reads E); relu alternates ACT/DVE."""
            psPall = pspool.tile([T, NP], F32, tag="psP", bufs=1)
            handles = {}
            for i in range(T // 2 + 1):
                if i < 16:
                    psA = a_w1(i, E_all, pspool)
                    handles[i] = a_relu(i, psA, on_act=(i % 2 == 0))
                if i >= 1:
                    a_w2(i - 1, handles.pop(i - 1), psPall)
            return psPall

        def phase_gate(psPall):
            """sigmoid(adj + b2) * fmask -> gate; stage to DRAM for the
            partition-broadcast reads of Phase M."""
            gsig = gpool.tile([T, NP], F32, tag="gsig")
            nc.scalar.activation(gsig[:], psPall[:], FR.Sigmoid,
                                 bias=b2c_s)
            gate_cur = gpool.tile([T, NP], dt, tag="gate")
            nc.vector.tensor_mul(gate_cur[:], gsig[:], fmaskp_s)
            gate_dram = gdram.tile([T, NP], dt, tag="gdr")
            nc.sync.dma_start(gate_dram[:], gate_cur[:])
            return gate_dram

        def u_prepare(pspool, r):
            """U = Wh @ h for all 32 frames in one 768-col matmul (vs 18432
            broadcast cols on the PE); staged to SBUF for the stride-0
            broadcast-add in m_point. Runs during the prior GRU drain."""
            psU_all = pspool.tile([D, TN], F32, tag="psP", bufs=1,
                                  name="psUall")
            mm512(psU_all, wht_s, Hn16[:], start=True, stop=True)
            u_sb = apool.tile([D, TN], BF16, tag="usb", bufs=2)
            nc.scalar.activation(u_sb[:], psU_all[:], FR.Identity, bias=0.0)
            return u_sb

        def m_mm(tp, pspool):
            """psM = We@E for 2 frames (Wh part added by m_point via DVE)."""
            t0 = 2 * tp
            psM = pspool.tile([D, 2 * NP], F32, tag="big", bufs=2)
            mm512(psM, wet_s, E_all[:, t0 * NP:(t0 + 2) * NP],
                  start=True, stop=True, base=0)
            return psM

        def gate_load(tp, gate_dram):
            t0 = 2 * tp
            gate_bc = bcpool.tile([D, 2 * NP], dt, tag="gbc")
            nc.sync.dma_start(
                gate_bc[:].rearrange("p (t n) -> p t n", t=2),
                gate_dram[t0:t0 + 2, :]
                .rearrange("(o t) n -> o t n", o=1).broadcast_to([D, 2, NP]))
            return gate_bc

        def m_point(tp, psM, gate_dram, on_act, u_sb):
            """hE = gate * relu(psM + Wh@h_w + b) for 2 frames; the Wh term
            is a stride-0 broadcast-add over the v axis on DVE."""
            t0 = 2 * tp
            gate_bc = gate_load(tp, gate_dram)
            hE = hE_all[:, t0 * NP:(t0 + 2) * NP]
            u_bc = u_sb[:, t0 * N:(t0 + 2) * N] \
                .rearrange("p (t o w) -> p t o w", t=2, o=1) \
                .broadcast_to([D, 2, N, N])
            psMv = psM[:].rearrange("p (t v w) -> p t v w", t=2, w=N)
            nc.vector.tensor_tensor(psMv, psMv, u_bc, op=ALU.add)
            if use_msg_bias or on_act:
                msg_sb = apool.tile([D, 2 * NP], dt, tag="msg")
                nc.scalar.activation(msg_sb[:], psM[:], FR.Relu,
                                     bias=msgbc_s)
                nc.vector.tensor_mul(hE, msg_sb[:], gate_bc[:])
            else:
                # hE = relu(psM) * gate  (exact: gate >= 0; msg_b == 0)
                nc.vector.scalar_tensor_tensor(
                    hE, psM[:], 0.0, gate_bc[:],
                    op0=ALU.max, op1=ALU.mult)

        def m_reduce(tp0, ntiles=2):
            """m_v for 2*ntiles frames: gpsimd folds w 24->12->6, DVE
            reduces the remaining 6."""
            nf = 2 * ntiles
            c0 = 2 * tp0 * NP
            hv = hE_all[:, c0:c0 + nf * NP] \
                .rearrange("p (v w) -> p v w", w=N)
            tr12 = trpool.tile([D, nf * N, N // 2], BF16, tag="tr")
            nc.gpsimd.tensor_add(tr12[:], hv[:, :, 0:N // 2],
                                 hv[:, :, N // 2:N])
            tr6 = trpool.tile([D, nf * N, N // 4], BF16, tag="tr6")
            nc.gpsimd.tensor_add(tr6[:], tr12[:, :, 0:N // 4],
                                 tr12[:, :, N // 4:N // 2])
            with nc.allow_low_precision("mv accum to bf16"):
                nc.vector.tensor_reduce(
                    mv_all[:, 2 * tp0 * N:(2 * tp0 + nf) * N], tr6[:],
                    axis=AX.X, op=ALU.add)

        def fused_m_a(r, pspool, gate_dram, psPall_next, u_sb):
            """Software-pipelined M_r fused with A_{r+1} (A consumes the
            hE tiles produced by m_point one iteration earlier)."""
            psMs, asbs = {}, {}
            for i in range(T // 2 + 2):
                if i < 16:
                    psMs[i] = m_mm(i, pspool)
                    m_point(i, psMs.pop(i), gate_dram,
                            on_act=(i % 2 == 1), u_sb=u_sb)
                if 1 <= i <= 16:
                    psA = a_w1(i - 1, hE_all, pspool)
                    asbs[i - 1] = a_relu(i - 1, psA, on_act=True)
                if 2 <= i <= 17:
                    a_w2(i - 2, asbs.pop(i - 2), psPall_next)
                if i % 2 == 1 and i < 16:
                    m_reduce(i - 1)

        def phase_gru_gen(pspool, half):
            """Batched GRU on [128, 768] (rounds 0/1, drained after the
            fused loop). Uses the psP psum slot so the next round's m_mm
            matmuls (big slots) overlap the drain via address deps."""
            cs = slice(0, TN)
            mv_h, hn16_h = mv_all[:, cs], Hn16[:, cs]
            H = TN
            psR = pspool.tile([D, H], F32, tag="psP", bufs=1)
            mm512(psR, gwihr_s, mv_h, start=True)
            mm512(psR, gwhhr_s, hn16_h, start=False, stop=True)
            r_g = utpool.tile([D, H], F32, tag=f"r_g{half}")
            nc.scalar.activation(r_g[:], psR[:], FR.Sigmoid,
                                 bias=gbr_s)
            yield

            psHN = pspool.tile([D, H], F32, tag="psP", bufs=1)
            mm512(psHN, gwhhn_s, hn16_h, start=True, stop=True)
            t2 = utpool.tile([D, H], F32, tag=f"t2{half}")
            nc.vector.scalar_tensor_tensor(
                t2[:], psHN[:], gbhn_s, r_g[:],
                op0=ALU.add, op1=ALU.mult)
            yield

            psIN = pspool.tile([D, H], F32, tag="psP", bufs=1)
            mm512(psIN, gwihn_s, mv_h, start=True, stop=True)
            nc.vector.scalar_tensor_tensor(
                t2[:], psIN[:], gbin_s, t2[:],
                op0=ALU.add, op1=ALU.add)
            n_g = utpool.tile([D, H], F32, tag=f"n_g{half}")
            nc.scalar.activation(n_g[:], t2[:], FR.Tanh, bias=0.0)
            yield

            psZ = pspool.tile([D, H], F32, tag="psP", bufs=1)
            mm512(psZ, gwihz_s, mv_h, start=True)
            mm512(psZ, gwhhz_s, hn16_h, start=False, stop=True)
            z_g = utpool.tile([D, H], F32, tag=f"z_g{half}")
            nc.scalar.activation(z_g[:], psZ[:], FR.Sigmoid,
                                 bias=gbz_s)

            # h_new = mask * (n + z*(h - n)); bf16 h state throughout
            nc.vector.tensor_sub(t2[:], hn16_h, n_g[:])
            nc.vector.tensor_mul(t2[:], t2[:], z_g[:])
            nc.vector.tensor_add(t2[:], t2[:], n_g[:])
            nc.vector.tensor_mul(hn16_h, t2[:], masknf_s[:, cs])
            yield

        # ---------------- round-2 fused M + GRU + LSTM + readout ---------
        FCH = 8            # frames per GRU/LSTM chunk
        CCOLS = FCH * N    # 192 state cols per chunk

        def gru_chunk(poolB, c0, nf):
            """GRU for frames c0..c0+nf-1; quantities spaced nf*32 cols so
            nf=8 spans 2 banks (start=True on r and hn), nf=4 one bank
            (start=True on r only)."""
            cc = nf * N
            sp = nf * 32
            cs = slice(c0 * N, c0 * N + cc)
            mv_h, hn_h = mv_all[:, cs], Hn16[:, cs]
            psU = poolB.tile([D, 4 * sp], F32, tag="psGRU", bufs=1,
                             name="psU")
            nc.tensor.matmul(psU[:, 0:cc], gwihr_s, mv_h,
                             start=True, stop=False)
            nc.tensor.matmul(psU[:, 0:cc], gwhhr_s, hn_h,
                             start=False, stop=True)
            nc.tensor.matmul(psU[:, sp:sp + cc], gwihz_s, mv_h,
                             start=False, stop=False)
            nc.tensor.matmul(psU[:, sp:sp + cc], gwhhz_s, hn_h,
                             start=False, stop=True)
            nc.tensor.matmul(psU[:, 2 * sp:2 * sp + cc], gwhhn_s, hn_h,
                             start=(nf == 8), stop=True)
            nc.tensor.matmul(psU[:, 3 * sp:3 * sp + cc], gwihn_s, mv_h,
                             start=False, stop=True)
            r_g = utpool.tile([D, cc], F32, tag="cr_g", bufs=2)
            nc.scalar.activation(r_g[:], psU[:, 0:cc], FR.Sigmoid,
                                 bias=gbr_s)
            t2 = utpool.tile([D, cc], F32, tag="ct2", bufs=2)
            nc.vector.scalar_tensor_tensor(
                t2[:], psU[:, 2 * sp:2 * sp + cc], gbhn_s, r_g[:],
                op0=ALU.add, op1=ALU.mult)
            nc.vector.scalar_tensor_tensor(
                t2[:], psU[:, 3 * sp:3 * sp + cc], gbin_s, t2[:],
                op0=ALU.add, op1=ALU.add)
            n_g = utpool.tile([D, cc], F32, tag="cn_g", bufs=2)
            nc.scalar.activation(n_g[:], t2[:], FR.Tanh, bias=0.0)
            z_g = utpool.tile([D, cc], F32, tag="cz_g", bufs=2)
            nc.scalar.activation(z_g[:], psU[:, sp:sp + cc],
                                 FR.Sigmoid, bias=gbz_s)
            # h_new trio on gpsimd: off the LSTM critical chain, keeps DVE
            # (the region's scarcest engine) thin
            nc.gpsimd.tensor_sub(t2[:], hn_h, n_g[:])
            nc.gpsimd.tensor_mul(t2[:], t2[:], z_g[:])
            nc.gpsimd.tensor_add(t2[:], t2[:], n_g[:])
            nc.vector.tensor_mul(hn_h, t2[:], masknf_s[:, cs])

        def wih_chunk(poolB, c0, nf):
            """Wih @ h_gnn for frames c0..c0+nf-1 into psG [4g, nf, 32pad]."""
            psG = poolB.tile([D, 4 * nf * 32], F32, tag="psG", bufs=1,
                             name="psG")
            v = psG.rearrange("p (g f s) -> p g f s", g=4, s=32)
            rhs = Hn16[:, c0 * N:(c0 + nf) * N]
            for g in range(4):
                nc.tensor.matmul(v[:, g:g + 1, :, 0:N],
                                 lwih_s[:, g * 128:(g + 1) * 128], rhs,
                                 start=(g % 2 == 0) if nf == 8 else (g == 0),
                                 stop=False)
            return v

        def lstm_step(t, psG_v, base):
            """One LSTM step; gate order i,f,o,g with g pre-scaled by 2:
            tanh(g) == 2*sig(2g) - 1 folded into the DVE ops."""
            t8 = t - base
            if t > 0:
                h_prev = Hout_all[:, (t - 1) * N:t * N]
                for g in range(4):
                    nc.tensor.matmul(
                        psG_v[:, g:g + 1, t8:t8 + 1, 0:N],
                        lwhh_s[:, g * 128:(g + 1) * 128], h_prev,
                        start=False, stop=True)
            sig = lpool.tile([D, 96], F32, tag="sig", bufs=3)
            if use_lstm_bias:
                for g in range(4):
                    nc.scalar.activation(
                        sig[:, 24 * g:24 * (g + 1)],
                        psG_v[:, g, t8:t8 + 1, 0:N], FR.Sigmoid,
                        bias=lb4_s[:, g:g + 1])
            else:
                nc.scalar.activation(
                    sig[:].rearrange("p (g o w) -> p g o w", g=4, o=1),
                    psG_v[:, 0:4, t8:t8 + 1, 0:N], FR.Sigmoid, bias=0.0)
            s_i, s_f = sig[:, 0:24], sig[:, 24:48]
            s_o, s_g2 = sig[:, 48:72], sig[:, 72:96]
            tmpa = lpool.tile([D, N], F32, tag="tmpa", bufs=2)
            # tmpa = (sig(2g) - 0.5) * sig(i);  i*tanh(g) == 2*tmpa
            nc.vector.scalar_tensor_tensor(
                tmpa[:], s_g2, -0.5, s_i, op0=ALU.add, op1=ALU.mult)
            if t == 0:
                nc.vector.tensor_scalar(c_sb[:], tmpa[:], 2.0, 0.0,
                                        op0=ALU.mult, op1=ALU.add)
            else:
                # m1 = c*sig(f) on gpsimd: it has ~1us of slack behind the
                # tanh/h/Whh/sig hops, so gpsimd latency is tolerable
                m1 = lpool.tile([D, N], F32, tag="m1", bufs=2)
                nc.gpsimd.tensor_mul(m1[:], c_sb[:], s_f)
                nc.vector.scalar_tensor_tensor(
                    c_sb[:], tmpa[:], 2.0, m1[:], op0=ALU.mult, op1=ALU.add)
            tcs = lpool.tile([D, N], F32, tag="tcs", bufs=2)
            nc.scalar.activation(tcs[:], c_sb[:], FR.Tanh, bias=0.0)
            nc.vector.tensor_mul(Hout_all[:, t * N:(t + 1) * N], s_o, tcs[:])

        def readout_chunk(poolB, half):
            cs = slice(half * 384, (half + 1) * 384)
            psRO = poolB.tile([C, 384], F32, tag="psG", bufs=1,
                              name="psRO")
            nc.tensor.matmul(psRO[:, 0:384], rowt_s, Hout_all[:, cs],
                             start=True, stop=True)
            pr = lpool.tile([C, 384], F32, tag="pr", bufs=2)
            nc.scalar.activation(pr[:], psRO[:], FR.Identity, bias=rob_s)
            nc.vector.tensor_mul(pr[:], pr[:], maskro_s[:, cs])
            nc.sync.dma_start(pred[:, cs], pr[:])

        def fused_m_lstm(poolB, gate_dram):
            """Round-2 M phase (single-frame tiles, psM double-buffered)
            with GRU chunks + woven LSTM + readout."""
            st = {"t": 0, "avail": 0, "psG_v": None, "base": 0}

            def emit_steps(n):
                while n > 0 and st["t"] < st["avail"]:
                    lstm_step(st["t"], st["psG_v"], st["base"])
                    st["t"] += 1
                    n -= 1
                    if st["t"] == 16:
                        readout_chunk(poolB, 0)

            for i in range(T):
                # --- one-frame M tile ---
                psM = poolB.tile([D, NP], F32, tag="psM2", bufs=2,
                                 name="psM")
                mm512(psM, wet_s, E_all[:, i * NP:(i + 1) * NP],
                      start=True, base=0)
                mh_mms(psM, 0, Hn16[:, i * N:(i + 1) * N])
                gate_bc = bcpool.tile([D, NP], dt, tag="gbc1", bufs=6)
                nc.sync.dma_start(
                    gate_bc[:].rearrange("p (t n) -> p t n", t=1),
                    gate_dram[i:i + 1, :]
                    .rearrange("(o t) n -> o t n", o=1)
                    .broadcast_to([D, 1, NP]))
                hE = hE_all[:, i * NP:(i + 1) * NP]
                if use_msg_bias or i % 2 == 1:
                    # odd tiles: ACT relu + DVE mul; even: fused DVE STT
                    msg_sb = apool.tile([D, NP], dt, tag="msg1", bufs=6)
                    nc.scalar.activation(msg_sb[:], psM[:], FR.Relu,
                                         bias=msgbc_s)
                    nc.vector.tensor_mul(hE, msg_sb[:], gate_bc[:])
                else:
                    nc.vector.scalar_tensor_tensor(
                        hE, psM[:], 0.0, gate_bc[:],
                        op0=ALU.max, op1=ALU.mult)
                if i % 4 == 3:
                    m_reduce((i - 3) // 2)
                # chunks 8,8,8,4,4: the two final 4-frame chunks unlock the
                # last LSTM steps before the loop's end (shorter tail)
                if i in (7, 15, 23, 27, 31):
                    c0, nf = {7: (0, 8), 15: (8, 8), 23: (16, 8),
                              27: (24, 4), 31: (28, 4)}[i]
                    gru_chunk(poolB, c0, nf)
                    st["psG_v"] = wih_chunk(poolB, c0, nf)
                    st["base"] = c0
                    st["avail"] = c0 + nf
                if i % 2 == 1:
                    # blocks of 4 keep consecutive chain ops adjacent in
                    # the engine queues (fewer M ops wedged between hops)
                    emit_steps(4)
            while st["t"] < T:
                emit_steps(6)
            readout_chunk(poolB, 1)

        # ---------------- pipeline ----------------
        with ExitStack() as gnn_ctx:
            pspool = gnn_ctx.enter_context(
                tc.tile_pool(name="ps", bufs=1, space="PSUM"))
            psPall = phase_a0(pspool)
            nc.vector.tensor_copy(Hn16[:], Hn_all[:])
            gate_dram = phase_gate(psPall)
            for r in range(P_ROUNDS - 1):
                u_sb = u_prepare(pspool, r)
                psPall_next = pspool.tile([T, NP], F32, tag="psP", bufs=1)
                fused_m_a(r, pspool, gate_dram, psPall_next, u_sb)
                gate_dram = phase_gate(psPall_next)
                g = phase_gru_gen(pspool, half=0)
                for _ in range(4):
                    next(g, None)

        with ExitStack() as l_ctx:
            poolB = l_ctx.enter_context(
                tc.tile_pool(name="psB", bufs=1, space="PSUM"))
            fused_m_lstm(poolB, gate_dram)

    nc.compile()
    return nc


def _prep_inputs(inputs):
    node_resnet = np.asarray(inputs["node_resnet"], np.float32)
    edge_resnet = np.asarray(inputs["edge_resnet"], np.float32)
    node_num = np.asarray(inputs["node_num_rec"]).astype(np.int64)

    nmask = (np.arange(N)[None, None, :] < node_num[:, :, None])  # [B,T,N]
    pmask = (nmask[:, :, :, None] & nmask[:, :, None, :])         # [B,T,N,N]

    w = {k: np.asarray(v, np.float32) for k, v in inputs.items()
         if k not in ("node_resnet", "edge_resnet", "node_num_rec")}

    lWih = w["lstm_Wih"].reshape(4, H_LSTM, D)
    lWhh = w["lstm_Whh"].reshape(4, H_LSTM, H_LSTM)
    lb = (w["lstm_bih"] + w["lstm_bhh"]).reshape(4, H_LSTM)
    perm = [0, 1, 3, 2]  # i,f,g,o -> i,f,o,g
    lWih, lWhh, lb = lWih[perm].copy(), lWhh[perm].copy(), lb[perm].copy()
    # pre-scale the g gate by 2: tanh(x) = 2*sigmoid(2x) - 1
    lWih[3] *= 2.0
    lWhh[3] *= 2.0
    lb[3] *= 2.0
    lwih_t = np.concatenate([lWih[g].T for g in range(4)], axis=1)
    lwhh_t = np.concatenate([lWhh[g].T for g in range(4)], axis=1)

    gWih = w["gru_Wih"].reshape(3, D, D)
    gWhh = w["gru_Whh"].reshape(3, D, D)
    gbih = w["gru_bih"].reshape(3, D)
    gbhh = w["gru_bhh"].reshape(3, D)

    f32c = lambda x: np.ascontiguousarray(np.asarray(x, np.float32))

    flags = []
    if np.any(lb != 0):
        flags.append("lstm_bias")
    if np.any(w["msg_b"] != 0):
        flags.append("msg_bias")

    bf16_items = {
        "w1t": w["link_W1"].T,
        "w2pad": np.concatenate(
            [np.zeros((D, 32), np.float32),
             w["link_W2"].T.reshape(D, 1),
             np.zeros((D, 31), np.float32)], axis=1),
        "wet": w["msg_We"].T,
        "wht": w["msg_Wh"].T,
        "gwihr": gWih[0].T, "gwihz": gWih[1].T, "gwihn": gWih[2].T,
        "gwhhr": gWhh[0].T, "gwhhz": gWhh[1].T, "gwhhn": gWhh[2].T,
        "lwih": lwih_t, "lwhh": lwhh_t,
        "rowt": w["ro_W"].T,
    }
    cbund = np.zeros((D, BF16_COLS), BULK_NP)
    for name, (o, wd) in BF16_OFF.items():
        arr = np.asarray(bf16_items[name], np.float32)
        cbund[:arr.shape[0], o:o + wd] = arr.astype(BULK_NP)

    f32_common = {
        "b1c": w["link_b1"].reshape(D, 1),
        "b2c": np.full((T, 1), w["link_b2"][0], np.float32),
        "msgbc": w["msg_b"].reshape(D, 1),
        "gbr": (gbih[0] + gbhh[0]).reshape(D, 1),
        "gbz": (gbih[1] + gbhh[1]).reshape(D, 1),
        "gbin": gbih[2].reshape(D, 1),
        "gbhn": gbhh[2].reshape(D, 1),
        "lb4": lb.T,
        "rob": w["ro_b"].reshape(C, 1),
    }

    in_maps = []
    for b in range(B):
        e = edge_resnet[b].reshape(T, D, NP).transpose(1, 0, 2)
        nd = node_resnet[b].transpose(1, 0, 2).reshape(D, TN)
        fm = pmask[b].reshape(T, NP).astype(np.float32)
        mn = nmask[b].reshape(1, TN).astype(np.float32)
        fbund = np.zeros((D, F32_COLS), np.float32)
        for name, arr in f32_common.items():
            o, wd = F32_OFF[name]
            fbund[:arr.shape[0], o:o + wd] = arr
        mbund = np.zeros((D, F32M_COLS), np.float32)
        for name, arr in (("fmaskp", fm),
                          ("masknf", np.broadcast_to(mn, (D, TN))),
                          ("maskro", np.broadcast_to(mn, (C, TN)))):
            o, wd = F32M_OFF[name]
            mbund[:arr.shape[0], o:o + wd] = arr
        m = {
            "edge": _np_bulk(e.reshape(D, TNP)),
            "node": f32c(nd),
            "cbund": np.ascontiguousarray(cbund),
            "fbund": np.ascontiguousarray(fbund),
            "mbund": np.ascontiguousarray(mbund),
        }
        in_maps.append(m)
    return in_maps, tuple(flags)


def _get_prog(flags=()):
    key = tuple(flags)
    if key not in _PROG_CACHE:
        _PROG_CACHE[key] = _build_program(key)
    return _PROG_CACHE[key]


def run_cores(inputs, **kw):
    in_maps, flags = _prep_inputs(inputs)
    nc = _get_prog(flags)
    return run_bass_kernel_spmd(nc, in_maps, list(range(B)), **kw)


def kernel(**inputs) -> np.ndarray:
    res = run_cores(inputs)
    out = np.zeros((B, T, N, C), np.float32)
    for b in range(B):
        pr = np.asarray(res.results[b]["pred"], np.float32)
        out[b] = pr.reshape(C, T, N).transpose(1, 2, 0)
    return out


if __name__ == "__main__":
    _get_prog()
    print("program built OK")


# revision 38
# speedup vs baseline: 1.1163x; 1.1163x over previous
"""Trainium2 Bass kernel for nn_HGNN_lstm (GNN message passing + LSTM).

Sharding: data-parallel over batch B=8 across 8 NeuronCores (one video per
core, zero collectives). Small weights replicated.

Math notes (exploits guaranteed input structure from setup_inputs):
  - edge_resnet is zero at invalid pairs, node_resnet zero at invalid nodes.
    Hence with gate >= 0 (sigmoid * mask):
      h_edge_{r+1} = where(pair_mask, gate*Msg, edge) == gate * Msg
      h_node_{r+1} = where(node_mask, h_new, node)    == node_mask * h_new
  - FORM-1 recurrence: materialize hE = gate*Msg directly (it is needed for
    m_v anyway). Phase A then consumes hE with no gate recurrence at all:
      adj_r = W2 @ relu(W1 @ hE_{r-1}) + b2      (exact: relu passes gate)
  - LSTM g-gate weights are pre-scaled by 2 on host so tanh(g) = 2*sig(2g)-1
    comes from the same single 4-gate sigmoid ACT op (exact identity).
  - The final GNN round is fused with the temporal LSTM: the GRU runs in
    8-frame chunks as soon as m_v for those frames lands, and LSTM steps are
    woven ~2 per M-tile iteration so the serial (~1.5us/step) LSTM chain
    hides under the round-2 message-passing compute.
"""

import sys
from contextlib import ExitStack

import numpy as np

sys.path.insert(0, "/opt/trn_rl_repo")

import concourse.bacc as bacc  # noqa: E402
import concourse.bass as bass  # noqa: E402
import concourse.mybir as mybir  # noqa: E402
import concourse.tile as tile  # noqa: E402
from concourse.bass_utils import run_bass_kernel_spmd  # noqa: E402

B, T, N, D = 8, 32, 24, 128
H_LINK, H_LSTM, C, P_ROUNDS = 128, 128, 6, 3
NP = N * N  # 576 pairs per frame
TNP = T * NP  # 18432
TN = T * N  # 768

F32 = mybir.dt.float32
BF16 = mybir.dt.bfloat16
FR = mybir.ActivationFunctionType
ALU = mybir.AluOpType
AX = mybir.AxisListType

import ml_dtypes  # noqa: E402

BULK_DT = BF16
BULK_NP = ml_dtypes.bfloat16


def _np_bulk(x):
    return np.ascontiguousarray(np.asarray(x).astype(BULK_NP))


# Const bundles: one bf16 + one f32 [128, cols] tensor, loaded with a single
# DMA trigger each (23 separate const DMAs queued the first weight behind
# ~1.5MB of edge chunks and delayed the first matmul to ~23us).
BF16_SPEC = [
    ("w1t", 128), ("w2pad", 64), ("wet", 128), ("wht", 128),
    ("gwihr", 128), ("gwihz", 128), ("gwihn", 128),
    ("gwhhr", 128), ("gwhhz", 128), ("gwhhn", 128),
    ("lwih", 512), ("lwhh", 512), ("rowt", 6),
]
F32_SPEC = [
    ("b1c", 1), ("b2c", 1), ("msgbc", 1), ("gbr", 1), ("gbz", 1),
    ("gbin", 1), ("gbhn", 1), ("lb4", 4), ("rob", 1),
]
F32M_SPEC = [
    ("fmaskp", 576), ("masknf", 768), ("maskro", 768),
]


def _spec_offsets(spec):
    out, o = {}, 0
    for name, w in spec:
        out[name] = (o, w)
        o += w
    return out, o


BF16_OFF, BF16_COLS = _spec_offsets(BF16_SPEC)
F32_OFF, F32_COLS = _spec_offsets(F32_SPEC)
F32M_OFF, F32M_COLS = _spec_offsets(F32M_SPEC)

_PROG_CACHE = {}


def _build_program(flags=()):
    use_lstm_bias = "lstm_bias" in flags
    use_msg_bias = "msg_bias" in flags

    nc = bacc.Bacc("TRN2", target_bir_lowering=False, debug=False)
    dt = BULK_DT

    def din(name, shape, d=dt):
        return nc.dram_tensor(name, shape, d, kind="ExternalInput").ap()

    edge = din("edge", [D, TNP])          # [d, t*576 + v*24 + w]
    node = din("node", [D, TN], F32)      # [d, t*24 + n]
    cbund = din("cbund", [D, BF16_COLS])  # bf16 const bundle
    fbund = din("fbund", [D, F32_COLS], F32)   # small f32 consts
    mbund = din("mbund", [D, F32M_COLS], F32)  # masks (late load)

    pred = nc.dram_tensor("pred", [C, TN], F32, kind="ExternalOutput").ap()

    with tile.TileContext(nc) as tc, ExitStack() as ctx:
        cp = ctx.enter_context(tc.tile_pool(name="consts", bufs=1))
        _dma_engines = [nc.sync, nc.scalar, nc.gpsimd]
        _dma_rr = [0]

        def dma_rr(dst, src):
            eng = _dma_engines[_dma_rr[0] % len(_dma_engines)]
            _dma_rr[0] += 1
            eng.dma_start(dst, src)

        big = ctx.enter_context(tc.tile_pool(name="big", bufs=1))
        E_all = big.tile([D, TNP], dt)
        hE_all = big.tile([D, TNP], dt)
        Hn_all = big.tile([D, TN], F32)
        Hn16 = big.tile([D, TN], BF16)
        mv_all = big.tile([D, TN], BF16)
        Hout_all = big.tile([D, TN], BF16)
        c_sb = big.tile([D, N], F32)

        cb = cp.tile([D, BF16_COLS], dt, name="cb")
        fb = cp.tile([D, F32_COLS], F32, name="fb")
        mb = cp.tile([D, F32M_COLS], F32, name="mb")
        # Small bundles + node first, then the 4.7MB edge sweep; the 1MB
        # mask bundle last (first needed at gate_0, ~25us in).
        dma_rr(cb[:], cbund)
        dma_rr(fb[:], fbund)
        dma_rr(Hn_all[:], node)
        for c in range(16):
            sl = slice(c * (TNP // 16), (c + 1) * (TNP // 16))
            dma_rr(E_all[:, sl], edge[:, sl])
        dma_rr(mb[:], mbund)

        def cs_(name):
            o, w = BF16_OFF[name]
            return cb[:, o:o + w]

        def fs_(name, parts=D):
            o, w = F32_OFF[name]
            return fb[0:parts, o:o + w]

        def ms_(name, parts=D):
            o, w = F32M_OFF[name]
            return mb[0:parts, o:o + w]

        w1t_s = cs_("w1t")
        w2pad_s = cs_("w2pad")
        wet_s = cs_("wet")
        wht_s = cs_("wht")
        gwihr_s, gwihz_s, gwihn_s = cs_("gwihr"), cs_("gwihz"), cs_("gwihn")
        gwhhr_s, gwhhz_s, gwhhn_s = cs_("gwhhr"), cs_("gwhhz"), cs_("gwhhn")
        lwih_s = cs_("lwih")
        lwhh_s = cs_("lwhh")
        rowt_s = cs_("rowt")
        b1c_s = fs_("b1c")
        b2c_s = fs_("b2c", T)
        msgbc_s = fs_("msgbc")
        gbr_s, gbz_s = fs_("gbr"), fs_("gbz")
        gbin_s, gbhn_s = fs_("gbin"), fs_("gbhn")
        lb4_s = fs_("lb4")
        rob_s = fs_("rob", C)
        fmaskp_s = ms_("fmaskp", T)
        masknf_s = ms_("masknf")
        maskro_s = ms_("maskro", C)

        apool = ctx.enter_context(tc.tile_pool(name="apool", bufs=3))
        bcpool = ctx.enter_context(tc.tile_pool(name="bcpool", bufs=3))
        trpool = ctx.enter_context(tc.tile_pool(name="trpool", bufs=3))
        gpool = ctx.enter_context(tc.tile_pool(name="gpool", bufs=2))
        utpool = ctx.enter_context(tc.tile_pool(name="utpool", bufs=1))
        lpool = ctx.enter_context(tc.tile_pool(name="lpool", bufs=2))
        gdram = ctx.enter_context(
            tc.tile_pool(name="gdram", bufs=2, space="DRAM"))

        def mm512(out_ps, lhsT, rhs, start, stop=False, base=0):
            """Matmul split into <=512-col chunks aligned to PSUM banks."""
            nfree = rhs.shape[-1]
            o = 0
            while o < nfree:
                sz = min(512 - ((base + o) % 512), nfree - o)
                nc.tensor.matmul(out_ps[:, base + o:base + o + sz], lhsT,
                                 rhs[:, o:o + sz],
                                 start=start, stop=stop and (o + sz >= nfree))
                o += sz

        def mh_mms(ps, base, hn_t):
            """Accumulate Wh@h broadcast over v into ps[:, base:base+576]."""
            pos = 0
            while pos < 576:
                room = 512 - ((base + pos) % 512)
                take = min(room, 576 - pos)
                while take > 0:
                    v, wofs = divmod(pos, N)
                    if wofs == 0 and take >= N:
                        nv = take // N
                        rhs = hn_t.rearrange("p (o w) -> p o w", o=1) \
                                  .broadcast_to([D, nv, N])
                        adv = nv * N
                    else:
                        adv = min(take, N - wofs)
                        rhs = hn_t[:, wofs:wofs + adv]
                    nc.tensor.matmul(ps[:, base + pos:base + pos + adv],
                                     wht_s, rhs, start=False, stop=False)
                    pos += adv
                    take -= adv

        def a_w1(tp, Xr, pspool):
            t0 = 2 * tp
            psA = pspool.tile([D, 2 * NP], F32, tag="big", bufs=2)
            mm512(psA, w1t_s, Xr[:, t0 * NP:(t0 + 2) * NP],
                  start=True, stop=True, base=0)
            return psA

        def a_relu(tp, psA, on_act):
            a_sb = apool.tile([D, 2 * NP], dt, tag="a")
            if on_act:
                nc.scalar.activation(a_sb[:], psA[:], FR.Relu,
                                     bias=b1c_s)
            else:
                nc.vector.tensor_scalar(a_sb[:], psA[:], b1c_s, 0.0,
                                        op0=ALU.add, op1=ALU.max)
            return a_sb

        def a_w2(tp, a_sb, psPall):
            t0 = 2 * tp
            for k in range(2):
                t = t0 + k
                mm512(psPall, w2pad_s[:, 32 - t:64 - t],
                      a_sb[:, k * NP:(k + 1) * NP],
                      start=(t == 0), stop=(t == T - 1))

        def phase_a0(pspool):
            """Standalone A_0 (reads E); relu alternates ACT/DVE."""
            psPall = pspool.tile([T, NP], F32, tag="psP", bufs=1)
            handles = {}
            for i in range(T // 2 + 1):
                if i < 16:
                    psA = a_w1(i, E_all, pspool)
                    handles[i] = a_relu(i, psA, on_act=(i % 2 == 0))
                if i >= 1:
                    a_w2(i - 1, handles.pop(i - 1), psPall)
            return psPall

        def phase_gate(psPall):
            """sigmoid(adj + b2) * fmask -> gate; stage to DRAM for the
            partition-broadcast reads of Phase M."""
            gsig = gpool.tile([T, NP], F32, tag="gsig")
            nc.scalar.activation(gsig[:], psPall[:], FR.Sigmoid,
                                 bias=b2c_s)
            gate_cur = gpool.tile([T, NP], dt, tag="gate")
            nc.vector.tensor_mul(gate_cur[:], gsig[:], fmaskp_s)
            gate_dram = gdram.tile([T, NP], dt, tag="gdr")
            nc.sync.dma_start(gate_dram[:], gate_cur[:])
            return gate_dram

        def u_prepare(pspool, r):
            """U = Wh @ h for all 32 frames in one 768-col matmul (vs 18432
            broadcast cols on the PE); staged to SBUF for the stride-0
            broadcast-add in m_point. Runs during the prior GRU drain."""
            psU_all = pspool.tile([D, TN], F32, tag="psP", bufs=1,
                                  name="psUall")
            mm512(psU_all, wht_s, Hn16[:], start=True, stop=True)
            u_sb = apool.tile([D, TN], BF16, tag="usb", bufs=2)
            nc.scalar.activation(u_sb[:], psU_all[:], FR.Identity, bias=0.0)
            return u_sb

        def m_mm(tp, pspool):
            """psM = We@E for 2 frames. Hybrid Wh placement: odd tiles get
            the PE broadcast (mh_mms), even tiles get the DVE stride-0 add
            in m_point -- balances the two engines in rounds 0/1."""
            t0 = 2 * tp
            psM = pspool.tile([D, 2 * NP], F32, tag="big", bufs=2)
            hybrid_mh = (tp % 2 == 1)
            mm512(psM, wet_s, E_all[:, t0 * NP:(t0 + 2) * NP],
                  start=True, stop=not hybrid_mh, base=0)
            if hybrid_mh:
                for k in range(2):
                    mh_mms(psM, k * NP,
                           Hn16[:, (t0 + k) * N:(t0 + k + 1) * N])
            return psM

        def gate_load(tp, gate_dram):
            t0 = 2 * tp
            gate_bc = bcpool.tile([D, 2 * NP], dt, tag="gbc")
            nc.sync.dma_start(
                gate_bc[:].rearrange("p (t n) -> p t n", t=2),
                gate_dram[t0:t0 + 2, :]
                .rearrange("(o t) n -> o t n", o=1).broadcast_to([D, 2, NP]))
            return gate_bc

        def m_point(tp, psM, gate_dram, on_act, u_sb):
            """hE = gate * relu(psM + Wh@h_w + b) for 2 frames; the Wh term
            is a stride-0 broadcast-add over the v axis on DVE."""
            t0 = 2 * tp
            gate_bc = gate_load(tp, gate_dram)
            hE = hE_all[:, t0 * NP:(t0 + 2) * NP]
            if tp % 2 == 0:
                u_bc = u_sb[:, t0 * N:(t0 + 2) * N] \
                    .rearrange("p (t o w) -> p t o w", t=2, o=1) \
                    .broadcast_to([D, 2, N, N])
                psMv = psM[:].rearrange("p (t v w) -> p t v w", t=2, w=N)
                nc.vector.tensor_tensor(psMv, psMv, u_bc, op=ALU.add)
            if use_msg_bias or on_act:
                msg_sb = apool.tile([D, 2 * NP], dt, tag="msg")
                nc.scalar.activation(msg_sb[:], psM[:], FR.Relu,
                                     bias=msgbc_s)
                nc.vector.tensor_mul(hE, msg_sb[:], gate_bc[:])
            else:
                # hE = relu(psM) * gate  (exact: gate >= 0; msg_b == 0)
                nc.vector.scalar_tensor_tensor(
                    hE, psM[:], 0.0, gate_bc[:],
                    op0=ALU.max, op1=ALU.mult)

        def m_reduce(tp0, ntiles=2):
            """m_v for 2*ntiles frames: gpsimd folds w 24->12->6, DVE
            reduces the remaining 6."""
            nf = 2 * ntiles
            c0 = 2 * tp0 * NP
            hv = hE_all[:, c0:c0 + nf * NP] \
                .rearrange("p (v w) -> p v w", w=N)
            tr12 = trpool.tile([D, nf * N, N // 2], BF16, tag="tr")
            nc.gpsimd.tensor_add(tr12[:], hv[:, :, 0:N // 2],
                                 hv[:, :, N // 2:N])
            tr6 = trpool.tile([D, nf * N, N // 4], BF16, tag="tr6")
            nc.gpsimd.tensor_add(tr6[:], tr12[:, :, 0:N // 4],
                                 tr12[:, :, N // 4:N // 2])
            with nc.allow_low_precision("mv accum to bf16"):
                nc.vector.tensor_reduce(
                    mv_all[:, 2 * tp0 * N:(2 * tp0 + nf) * N], tr6[:],
                    axis=AX.X, op=ALU.add)

        def fused_m_a(r, pspool, gate_dram, psPall_next, u_sb):
            """Software-pipelined M_r fused with A_{r+1} (A consumes the
            hE tiles produced by m_point one iteration earlier)."""
            psMs, asbs = {}, {}
            for i in range(T // 2 + 2):
                if i < 16:
                    psMs[i] = m_mm(i, pspool)
                    m_point(i, psMs.pop(i), gate_dram,
                            on_act=(i % 2 == 1), u_sb=u_sb)
                if 1 <= i <= 16:
                    psA = a_w1(i - 1, hE_all, pspool)
                    asbs[i - 1] = a_relu(i - 1, psA, on_act=True)
                if 2 <= i <= 17:
                    a_w2(i - 2, asbs.pop(i - 2), psPall_next)
                if i % 2 == 1 and i < 16:
                    m_reduce(i - 1)

        def phase_gru_gen(pspool, half):
            """Batched GRU on [128, 768] (rounds 0/1, drained after the
            fused loop). Uses the psP psum slot so the next round's m_mm
            matmuls (big slots) overlap the drain via address deps."""
            cs = slice(0, TN)
            mv_h, hn16_h = mv_all[:, cs], Hn16[:, cs]
            H = TN
            psR = pspool.tile([D, H], F32, tag="psP", bufs=1)
            mm512(psR, gwihr_s, mv_h, start=True)
            mm512(psR, gwhhr_s, hn16_h, start=False, stop=True)
            r_g = utpool.tile([D, H], F32, tag=f"r_g{half}")
            nc.scalar.activation(r_g[:], psR[:], FR.Sigmoid,
                                 bias=gbr_s)
            yield

            psHN = pspool.tile([D, H], F32, tag="psP", bufs=1)
            mm512(psHN, gwhhn_s, hn16_h, start=True, stop=True)
            t2 = utpool.tile([D, H], F32, tag=f"t2{half}")
            nc.vector.scalar_tensor_tensor(
                t2[:], psHN[:], gbhn_s, r_g[:],
                op0=ALU.add, op1=ALU.mult)
            yield

            psIN = pspool.tile([D, H], F32, tag="psP", bufs=1)
            mm512(psIN, gwihn_s, mv_h, start=True, stop=True)
            nc.vector.scalar_tensor_tensor(
                t2[:], psIN[:], gbin_s, t2[:],
                op0=ALU.add, op1=ALU.add)
            n_g = utpool.tile([D, H], F32, tag=f"n_g{half}")
            nc.scalar.activation(n_g[:], t2[:], FR.Tanh, bias=0.0)
            yield

            psZ = pspool.tile([D, H], F32, tag="psP", bufs=1)
            mm512(psZ, gwihz_s, mv_h, start=True)
            mm512(psZ, gwhhz_s, hn16_h, start=False, stop=True)
            z_g = utpool.tile([D, H], F32, tag=f"z_g{half}")
            nc.scalar.activation(z_g[:], psZ[:], FR.Sigmoid,
                                 bias=gbz_s)

            # h_new = mask * (n + z*(h - n)); bf16 h state throughout
            nc.vector.tensor_sub(t2[:], hn16_h, n_g[:])
            nc.vector.tensor_mul(t2[:], t2[:], z_g[:])
            nc.vector.tensor_add(t2[:], t2[:], n_g[:])
            nc.vector.tensor_mul(hn16_h, t2[:], masknf_s[:, cs])
            yield

        # ---------------- round-2 fused M + GRU + LSTM + readout ---------
        FCH = 8            # frames per GRU/LSTM chunk
        CCOLS = FCH * N    # 192 state cols per chunk

        def gru_chunk(poolB, c0, nf):
            """GRU for frames c0..c0+nf-1; quantities spaced nf*32 cols so
            nf=8 spans 2 banks (start=True on r and hn), nf=4 one bank
            (start=True on r only)."""
            cc = nf * N
            sp = nf * 32
            cs = slice(c0 * N, c0 * N + cc)
            mv_h, hn_h = mv_all[:, cs], Hn16[:, cs]
            psU = poolB.tile([D, 4 * sp], F32, tag="psGRU", bufs=1,
                             name="psU")
            nc.tensor.matmul(psU[:, 0:cc], gwihr_s, mv_h,
                             start=True, stop=False)
            nc.tensor.matmul(psU[:, 0:cc], gwhhr_s, hn_h,
                             start=False, stop=True)
            nc.tensor.matmul(psU[:, sp:sp + cc], gwihz_s, mv_h,
                             start=False, stop=False)
            nc.tensor.matmul(psU[:, sp:sp + cc], gwhhz_s, hn_h,
                             start=False, stop=True)
            nc.tensor.matmul(psU[:, 2 * sp:2 * sp + cc], gwhhn_s, hn_h,
                             start=(nf == 8), stop=True)
            nc.tensor.matmul(psU[:, 3 * sp:3 * sp + cc], gwihn_s, mv_h,
                             start=False, stop=True)
            r_g = utpool.tile([D, cc], F32, tag="cr_g", bufs=2)
            nc.scalar.activation(r_g[:], psU[:, 0:cc], FR.Sigmoid,
                                 bias=gbr_s)
            t2 = utpool.tile([D, cc], F32, tag="ct2", bufs=2)
            nc.vector.scalar_tensor_tensor(
                t2[:], psU[:, 2 * sp:2 * sp + cc], gbhn_s, r_g[:],
                op0=ALU.add, op1=ALU.mult)
            nc.vector.scalar_tensor_tensor(
                t2[:], psU[:, 3 * sp:3 * sp + cc], gbin_s, t2[:],
                op0=ALU.add, op1=ALU.add)
            n_g = utpool.tile([D, cc], F32, tag="cn_g", bufs=2)
            nc.scalar.activation(n_g[:], t2[:], FR.Tanh, bias=0.0)
            z_g = utpool.tile([D, cc], F32, tag="cz_g", bufs=2)
            nc.scalar.activation(z_g[:], psU[:, sp:sp + cc],
                                 FR.Sigmoid, bias=gbz_s)
            # h_new trio on gpsimd: off the LSTM critical chain, keeps DVE
            # (the region's scarcest engine) thin
            nc.gpsimd.tensor_sub(t2[:], hn_h, n_g[:])
            nc.gpsimd.tensor_mul(t2[:], t2[:], z_g[:])
            nc.gpsimd.tensor_add(t2[:], t2[:], n_g[:])
            nc.vector.tensor_mul(hn_h, t2[:], masknf_s[:, cs])

        def wih_chunk(poolB, c0, nf):
            """Wih @ h_gnn for frames c0..c0+nf-1 into psG [4g, nf, 32pad]."""
            psG = poolB.tile([D, 4 * nf * 32], F32, tag="psG", bufs=1,
                             name="psG")
            v = psG.rearrange("p (g f s) -> p g f s", g=4, s=32)
            rhs = Hn16[:, c0 * N:(c0 + nf) * N]
            for g in range(4):
                nc.tensor.matmul(v[:, g:g + 1, :, 0:N],
                                 lwih_s[:, g * 128:(g + 1) * 128], rhs,
                                 start=(g % 2 == 0) if nf == 8 else (g == 0),
                                 stop=False)
            return v

        def lstm_step(t, psG_v, base):
            """One LSTM step; gate order i,f,o,g with g pre-scaled by 2:
            tanh(g) == 2*sig(2g) - 1 folded into the DVE ops."""
            t8 = t - base
            if t > 0:
                h_prev = Hout_all[:, (t - 1) * N:t * N]
                for g in range(4):
                    nc.tensor.matmul(
                        psG_v[:, g:g + 1, t8:t8 + 1, 0:N],
                        lwhh_s[:, g * 128:(g + 1) * 128], h_prev,
                        start=False, stop=True)
            sig = lpool.tile([D, 96], F32, tag="sig", bufs=3)
            if use_lstm_bias:
                for g in range(4):
                    nc.scalar.activation(
                        sig[:, 24 * g:24 * (g + 1)],
                        psG_v[:, g, t8:t8 + 1, 0:N], FR.Sigmoid,
                        bias=lb4_s[:, g:g + 1])
            else:
                nc.scalar.activation(
                    sig[:].rearrange("p (g o w) -> p g o w", g=4, o=1),
                    psG_v[:, 0:4, t8:t8 + 1, 0:N], FR.Sigmoid, bias=0.0)
            s_i, s_f = sig[:, 0:24], sig[:, 24:48]
            s_o, s_g2 = sig[:, 48:72], sig[:, 72:96]
            tmpa = lpool.tile([D, N], F32, tag="tmpa", bufs=2)
            # tmpa = (sig(2g) - 0.5) * sig(i);  i*tanh(g) == 2*tmpa
            nc.vector.scalar_tensor_tensor(
                tmpa[:], s_g2, -0.5, s_i, op0=ALU.add, op1=ALU.mult)
            if t == 0:
                nc.vector.tensor_scalar(c_sb[:], tmpa[:], 2.0, 0.0,
                                        op0=ALU.mult, op1=ALU.add)
            else:
                # m1 = c*sig(f) on gpsimd: it has ~1us of slack behind the
                # tanh/h/Whh/sig hops, so gpsimd latency is tolerable
                m1 = lpool.tile([D, N], F32, tag="m1", bufs=2)
                nc.gpsimd.tensor_mul(m1[:], c_sb[:], s_f)
                nc.vector.scalar_tensor_tensor(
                    c_sb[:], tmpa[:], 2.0, m1[:], op0=ALU.mult, op1=ALU.add)
            tcs = lpool.tile([D, N], F32, tag="tcs", bufs=2)
            nc.scalar.activation(tcs[:], c_sb[:], FR.Tanh, bias=0.0)
            nc.vector.tensor_mul(Hout_all[:, t * N:(t + 1) * N], s_o, tcs[:])

        def readout_chunk(poolB, half):
            cs = slice(half * 384, (half + 1) * 384)
            psRO = poolB.tile([C, 384], F32, tag="psG", bufs=1,
                              name="psRO")
            nc.tensor.matmul(psRO[:, 0:384], rowt_s, Hout_all[:, cs],
                             start=True, stop=True)
            pr = lpool.tile([C, 384], F32, tag="pr", bufs=2)
            nc.scalar.activation(pr[:], psRO[:], FR.Identity, bias=rob_s)
            nc.vector.tensor_mul(pr[:], pr[:], maskro_s[:, cs])
            nc.sync.dma_start(pred[:, cs], pr[:])

        def fused_m_lstm(poolB, gate_dram):
            """Round-2 M phase (single-frame tiles, psM double-buffered)
            with GRU chunks + woven LSTM + readout."""
            st = {"t": 0, "avail": 0, "psG_v": None, "base": 0}

            def emit_steps(n):
                while n > 0 and st["t"] < st["avail"]:
                    lstm_step(st["t"], st["psG_v"], st["base"])
                    st["t"] += 1
                    n -= 1
                    if st["t"] == 16:
                        readout_chunk(poolB, 0)

            for i in range(T):
                # --- one-frame M tile ---
                psM = poolB.tile([D, NP], F32, tag="psM2", bufs=2,
                                 name="psM")
                mm512(psM, wet_s, E_all[:, i * NP:(i + 1) * NP],
                      start=True, base=0)
                mh_mms(psM, 0, Hn16[:, i * N:(i + 1) * N])
                gate_bc = bcpool.tile([D, NP], dt, tag="gbc1", bufs=6)
                nc.sync.dma_start(
                    gate_bc[:].rearrange("p (t n) -> p t n", t=1),
                    gate_dram[i:i + 1, :]
                    .rearrange("(o t) n -> o t n", o=1)
                    .broadcast_to([D, 1, NP]))
                hE = hE_all[:, i * NP:(i + 1) * NP]
                if use_msg_bias or i % 2 == 1:
                    # odd tiles: ACT relu + DVE mul; even: fused DVE STT
                    msg_sb = apool.tile([D, NP], dt, tag="msg1", bufs=6)
                    nc.scalar.activation(msg_sb[:], psM[:], FR.Relu,
                                         bias=msgbc_s)
                    nc.vector.tensor_mul(hE, msg_sb[:], gate_bc[:])
                else:
                    nc.vector.scalar_tensor_tensor(
                        hE, psM[:], 0.0, gate_bc[:],
                        op0=ALU.max, op1=ALU.mult)
                if i % 4 == 3:
                    m_reduce((i - 3) // 2)
                # chunks 8,8,8,4,4: the two final 4-frame chunks unlock the
                # last LSTM steps before the loop's end (shorter tail)
                if i in (7, 15, 23, 27, 31):
                    c0, nf = {7: (0, 8), 15: (8, 8), 23: (16, 8),
                              27: (24, 4), 31: (28, 4)}[i]
                    gru_chunk(poolB, c0, nf)
                    st["psG_v"] = wih_chunk(poolB, c0, nf)
                    st["base"] = c0
                    st["avail"] = c0 + nf
                if i % 2 == 1:
                    # blocks of 4 keep consecutive chain ops adjacent in
                    # the engine queues (fewer M ops wedged between hops)
                    emit_steps(4)
            while st["t"] < T:
                emit_steps(6)
            readout_chunk(poolB, 1)

        # ---------------- pipeline ----------------
        with ExitStack() as gnn_ctx:
            pspool = gnn_ctx.enter_context(
                tc.tile_pool(name="ps", bufs=1, space="PSUM"))
            psPall = phase_a0(pspool)
            nc.vector.tensor_copy(Hn16[:], Hn_all[:])
            gate_dram = phase_gate(psPall)
            for r in range(P_ROUNDS - 1):
                u_sb = u_prepare(pspool, r)
                psPall_next = pspool.tile([T, NP], F32, tag="psP", bufs=1)
                fused_m_a(r, pspool, gate_dram, psPall_next, u_sb)
                gate_dram = phase_gate(psPall_next)
                g = phase_gru_gen(pspool, half=0)
                for _ in range(4):
                    next(g, None)

        with ExitStack() as l_ctx:
            poolB = l_ctx.enter_context(
                tc.tile_pool(name="psB", bufs=1, space="PSUM"))
            fused_m_lstm(poolB, gate_dram)

    nc.compile()
    return nc


def _prep_inputs(inputs):
    node_resnet = np.asarray(inputs["node_resnet"], np.float32)
    edge_resnet = np.asarray(inputs["edge_resnet"], np.float32)
    node_num = np.asarray(inputs["node_num_rec"]).astype(np.int64)

    nmask = (np.arange(N)[None, None, :] < node_num[:, :, None])  # [B,T,N]
    pmask = (nmask[:, :, :, None] & nmask[:, :, None, :])         # [B,T,N,N]

    w = {k: np.asarray(v, np.float32) for k, v in inputs.items()
         if k not in ("node_resnet", "edge_resnet", "node_num_rec")}

    lWih = w["lstm_Wih"].reshape(4, H_LSTM, D)
    lWhh = w["lstm_Whh"].reshape(4, H_LSTM, H_LSTM)
    lb = (w["lstm_bih"] + w["lstm_bhh"]).reshape(4, H_LSTM)
    perm = [0, 1, 3, 2]  # i,f,g,o -> i,f,o,g
    lWih, lWhh, lb = lWih[perm].copy(), lWhh[perm].copy(), lb[perm].copy()
    # pre-scale the g gate by 2: tanh(x) = 2*sigmoid(2x) - 1
    lWih[3] *= 2.0
    lWhh[3] *= 2.0
    lb[3] *= 2.0
    lwih_t = np.concatenate([lWih[g].T for g in range(4)], axis=1)
    lwhh_t = np.concatenate([lWhh[g].T for g in range(4)], axis=1)

    gWih = w["gru_Wih"].reshape(3, D, D)
    gWhh = w["gru_Whh"].reshape(3, D, D)
    gbih = w["gru_bih"].reshape(3, D)
    gbhh = w["gru_bhh"].reshape(3, D)

    f32c = lambda x: np.ascontiguousarray(np.asarray(x, np.float32))

    flags = []
    if np.any(lb != 0):
        flags.append("lstm_bias")
    if np.any(w["msg_b"] != 0):
        flags.append("msg_bias")

    bf16_items = {
        "w1t": w["link_W1"].T,
        "w2pad": np.concatenate(
            [np.zeros((D, 32), np.float32),
             w["link_W2"].T.reshape(D, 1),
             np.zeros((D, 31), np.float32)], axis=1),
        "wet": w["msg_We"].T,
        "wht": w["msg_Wh"].T,
        "gwihr": gWih[0].T, "gwihz": gWih[1].T, "gwihn": gWih[2].T,
        "gwhhr": gWhh[0].T, "gwhhz": gWhh[1].T, "gwhhn": gWhh[2].T,
        "lwih": lwih_t, "lwhh": lwhh_t,
        "rowt": w["ro_W"].T,
    }
    cbund = np.zeros((D, BF16_COLS), BULK_NP)
    for name, (o, wd) in BF16_OFF.items():
        arr = np.asarray(bf16_items[name], np.float32)
        cbund[:arr.shape[0], o:o + wd] = arr.astype(BULK_NP)

    f32_common = {
        "b1c": w["link_b1"].reshape(D, 1),
        "b2c": np.full((T, 1), w["link_b2"][0], np.float32),
        "msgbc": w["msg_b"].reshape(D, 1),
        "gbr": (gbih[0] + gbhh[0]).reshape(D, 1),
        "gbz": (gbih[1] + gbhh[1]).reshape(D, 1),
        "gbin": gbih[2].reshape(D, 1),
        "gbhn": gbhh[2].reshape(D, 1),
        "lb4": lb.T,
        "rob": w["ro_b"].reshape(C, 1),
    }

    in_maps = []
    for b in range(B):
        e = edge_resnet[b].reshape(T, D, NP).transpose(1, 0, 2)
        nd = node_resnet[b].transpose(1, 0, 2).reshape(D, TN)
        fm = pmask[b].reshape(T, NP).astype(np.float32)
        mn = nmask[b].reshape(1, TN).astype(np.float32)
        fbund = np.zeros((D, F32_COLS), np.float32)
        for name, arr in f32_common.items():
            o, wd = F32_OFF[name]
            fbund[:arr.shape[0], o:o + wd] = arr
        mbund = np.zeros((D, F32M_COLS), np.float32)
        for name, arr in (("fmaskp", fm),
                          ("masknf", np.broadcast_to(mn, (D, TN))),
                          ("maskro", np.broadcast_to(mn, (C, TN)))):
            o, wd = F32M_OFF[name]
            mbund[:arr.shape[0], o:o + wd] = arr
        m = {
            "edge": _np_bulk(e.reshape(D, TNP)),
            "node": f32c(nd),
            "cbund": np.ascontiguousarray(cbund),
            "fbund": np.ascontiguousarray(fbund),
            "mbund": np.ascontiguousarray(mbund),
        }
        in_maps.append(m)
    return in_maps, tuple(flags)


def _get_prog(flags=()):
    key = tuple(flags)
    if key not in _PROG_CACHE:
        _PROG_CACHE[key] = _build_program(key)
    return _PROG_CACHE[key]


def run_cores(inputs, **kw):
    in_maps, flags = _prep_inputs(inputs)
    nc = _get_prog(flags)
    return run_bass_kernel_spmd(nc, in_maps, list(range(B)), **kw)


def kernel(**inputs) -> np.ndarray:
    res = run_cores(inputs)
    out = np.zeros((B, T, N, C), np.float32)
    for b in range(B):
        pr = np.asarray(res.results[b]["pred"], np.float32)
        out[b] = pr.reshape(C, T, N).transpose(1, 2, 0)
    return out


if __name__ == "__main__":
    _get_prog()
    print("program built OK")
